# revision 7
# baseline (speedup 1.0000x reference)
"""Fused per-token transformer block on 8 TRN2 NeuronCores.

Math (per token row x, o — no cross-token interaction anywhere):
  pred = relu(x@pw1+pb1)@pw2+pb2
  obs  = relu(o@ow1+ob1)@ow2+ob2
  res  = relu((pred-obs)@rw1+rb1)@rw2+rb2
  Q,K,V = x@wq+bq, x@wk+bk, x@wv+bv   (16 heads x 128)
  scores[h,g] = Q_h.(K_g/sqrt(128) - R_g);  attn = softmax_g
  att = (attn @ V) @ wo + bo
  y1 = LN(x+att)*g1+be1;  out = LN(y1 + lrelu(y1@fw1+fb1)@fw2+fb2)*g2+be2

Sharding: pure data-parallel over the 8192 token rows (1024/core), zero
collectives.  On-chip layout is "B-layout": features on partitions,
tokens on the free axis, so every projection uses the weight tile as the
stationary operand and activations as the moving operand with no
transposes.  Attention flips per 128-token tile to "A-layout" (tokens on
partitions) via PE transposes; scores/attn@V are DVE broadcast-multiply +
reduce.  LayerNorm stats are partition-dim sums via ones-matmuls.
"""

import math

import numpy as np
import ml_dtypes

import concourse.mybir as mybir
import concourse.tile as tile
from concourse import bacc
from concourse.bass_utils import run_bass_kernel_spmd
from concourse.masks import make_identity

AF = mybir.ActivationFunctionType
OP = mybir.AluOpType
AX = mybir.AxisListType
BF16 = mybir.dt.bfloat16
F32 = mybir.dt.float32

EPS = 1e-5
P = 128

CFG_FULL = dict(HID=2048, NH=16, FFN=8192, NTOK=1024, NPASS=2)
N_CORES = 8
N_ROWS = 8192


def build(cfg, debug=False):
    HID, NH, FFN, NTOK, NPASS = (
        cfg["HID"], cfg["NH"], cfg["FFN"], cfg["NTOK"], cfg["NPASS"])
    DH = HID // NH
    assert DH == P
    KT = HID // P          # feature tiles
    MT = KT
    NW = NTOK // NPASS     # tokens per pass
    TT = NW // P           # 128-token tiles per pass
    NJ = FFN // HID        # ffn column/row blocks
    MG = 4                 # m-tiles per psum wave

    nc = bacc.Bacc("TRN2", target_bir_lowering=False, debug=debug)

    embT_d = nc.dram_tensor("embT", (HID, NTOK), BF16, kind="ExternalInput")
    obsT_d = nc.dram_tensor("obsT", (HID, NTOK), BF16, kind="ExternalInput")
    wnames = ["pw1", "pw2", "ow1", "ow2", "rw1", "rw2", "wq", "wk", "wv", "wo"]
    wd = {n: nc.dram_tensor(n, (HID, HID), BF16, kind="ExternalInput")
          for n in wnames}
    fw1_d = nc.dram_tensor("fw1", (HID, FFN), BF16, kind="ExternalInput")
    fw2_d = nc.dram_tensor("fw2", (FFN, HID), BF16, kind="ExternalInput")
    bnames = ["pb1", "pb2", "ob1", "ob2", "rb1", "rb2", "bq", "bk", "bv", "bo",
              "fb2", "g1", "be1", "g2", "be2"]
    bd = {n: nc.dram_tensor(n, (P, MT), F32, kind="ExternalInput")
          for n in bnames}
    fb1a_d = nc.dram_tensor("fb1a", (P, MT * NJ), F32, kind="ExternalInput")
    fb1b_d = nc.dram_tensor("fb1b", (P, MT * NJ), F32, kind="ExternalInput")
    outT_d = nc.dram_tensor("outT", (HID, NTOK), F32, kind="ExternalOutput")

    with tile.TileContext(nc) as tc:
        with tc.tile_pool(name="constp", bufs=1) as constp, \
             tc.tile_pool(name="actp", bufs=88) as actp, \
             tc.tile_pool(name="f32p", bufs=24) as f32p, \
             tc.tile_pool(name="attp", bufs=2) as attp, \
             tc.tile_pool(name="attbig", bufs=1) as attbig, \
             tc.tile_pool(name="smallp", bufs=6) as smallp, \
             tc.tile_pool(name="wp", bufs=6) as wp, \
             tc.tile_pool(name="psmm", bufs=MG, space="PSUM") as psmm, \
             tc.tile_pool(name="pstr", bufs=2, space="PSUM") as pstr, \
             tc.tile_pool(name="psln", bufs=2, space="PSUM") as psln:

            ident = constp.tile([P, P], BF16, tag="ident", name="ident")
            make_identity(nc, ident[:])
            ones_col = constp.tile([P, 1], BF16, tag="ones_col", name="ones_col")
            nc.vector.memset(ones_col[:], 1.0)
            ones_row = constp.tile([1, P], F32, tag="ones_row", name="ones_row")
            nc.vector.memset(ones_row[:], 1.0)
            eps_t = constp.tile([1, 1], F32, tag="eps", name="eps")
            nc.vector.memset(eps_t[:], EPS)

            bias = {}
            for n in bnames:
                t = constp.tile([P, MT], F32, tag=f"b_{n}", name=f"b_{n}")
                nc.sync.dma_start(t[:], bd[n][:])
                bias[n] = t
            fb1a_t = constp.tile([P, MT * NJ], F32, tag="b_fb1a", name="b_fb1a")
            nc.sync.dma_start(fb1a_t[:], fb1a_d[:])
            fb1b_t = constp.tile([P, MT * NJ], F32, tag="b_fb1b", name="b_fb1b")
            nc.sync.dma_start(fb1b_t[:], fb1b_d[:])

            def new_set(tag="a", dtype=BF16, n=KT, pool=None):
                pool = pool or actp
                return [pool.tile([P, NW], dtype, tag=tag, name=tag) for _ in range(n)]

            def um(x_tiles, w_dram, w_row0, w_col0, n_ktiles, n_mtiles, evict):
                """out[m] = evict(m, sum_k W[k,m].T @ x[k]) in psum."""
                for mg0 in range(0, n_mtiles, MG):
                    mgn = min(MG, n_mtiles - mg0)
                    pss = [psmm.tile([P, NW], F32, tag="mm", name="mm") for _ in range(mgn)]
                    for k in range(n_ktiles):
                        wt = wp.tile([P, MG * P], BF16, tag="w", name="w")
                        nc.sync.dma_start(
                            wt[:, : mgn * P],
                            w_dram[w_row0 + k * P: w_row0 + (k + 1) * P,
                                   w_col0 + mg0 * P: w_col0 + (mg0 + mgn) * P])
                        for mi in range(mgn):
                            nc.tensor.matmul(
                                pss[mi][:],
                                wt[:, mi * P:(mi + 1) * P],
                                x_tiles[k][:],
                                start=(k == 0), stop=(k == n_ktiles - 1))
                    for mi in range(mgn):
                        evict(mg0 + mi, pss[mi])

            def act_evict(out_tiles, bias_t, bias_c0=0, func=AF.Identity, alpha=0.0):
                def ev(m, ps):
                    nc.scalar.activation(out_tiles[m][:], ps[:], func,
                                         bias=bias_t[:, bias_c0 + m: bias_c0 + m + 1],
                                         alpha=alpha)
                return ev

            def layernorm(x_tiles, g_t, be_t, out_tiles):
                ps_sum = psln.tile([1, NW], F32, tag="ln", name="ln")
                ps_sq = psln.tile([1, NW], F32, tag="ln", name="ln")
                for k in range(KT):
                    nc.tensor.matmul(ps_sum[0:1, :], ones_col[:], x_tiles[k][:],
                                     start=(k == 0), stop=(k == KT - 1))
                for k in range(KT):
                    sq = actp.tile([P, NW], BF16, tag="a", name="a")
                    nc.vector.tensor_mul(sq[:], x_tiles[k][:], x_tiles[k][:])
                    nc.tensor.matmul(ps_sq[0:1, :], ones_col[:], sq[:],
                                     start=(k == 0), stop=(k == KT - 1))
                mean = smallp.tile([1, NW], F32, tag="row", name="row")
                nc.scalar.activation(mean[0:1, :], ps_sum[0:1, :], AF.Copy,
                                     scale=1.0 / HID)
                msq = smallp.tile([1, NW], F32, tag="row", name="row")
                nc.scalar.activation(msq[0:1, :], ps_sq[0:1, :], AF.Copy,
                                     scale=1.0 / HID)
                var = smallp.tile([1, NW], F32, tag="row", name="row")
                nc.vector.tensor_mul(var[0:1, :], mean[0:1, :], mean[0:1, :])
                nc.vector.tensor_sub(var[0:1, :], msq[0:1, :], var[0:1, :])
                std = smallp.tile([1, NW], F32, tag="row", name="row")
                nc.scalar.activation(std[0:1, :], var[0:1, :], AF.Sqrt,
                                     bias=eps_t[0:1, 0:1])
                rstd = smallp.tile([1, NW], F32, tag="row", name="row")
                nc.vector.reciprocal(rstd[0:1, :], std[0:1, :])
                psb_m = psln.tile([P, NW], F32, tag="ln", name="ln")
                nc.tensor.matmul(psb_m[:], ones_row[0:1, :], mean[0:1, :],
                                 start=True, stop=True)
                psb_r = psln.tile([P, NW], F32, tag="ln", name="ln")
                nc.tensor.matmul(psb_r[:], ones_row[0:1, :], rstd[0:1, :],
                                 start=True, stop=True)
                for k in range(KT):
                    t1 = f32p.tile([P, NW], F32, tag="f", name="f")
                    nc.vector.tensor_sub(t1[:], x_tiles[k][:], psb_m[:])
                    nc.vector.tensor_mul(t1[:], t1[:], psb_r[:])
                    nc.vector.tensor_scalar(out_tiles[k][:], t1[:],
                                            g_t[:, k:k + 1], be_t[:, k:k + 1],
                                            op0=OP.mult, op1=OP.add)

            for p in range(NPASS):
                c0 = p * NW

                emb = new_set()
                obs = new_set()
                for k in range(KT):
                    nc.sync.dma_start(emb[k][:], embT_d[k * P:(k + 1) * P, c0:c0 + NW])
                    nc.sync.dma_start(obs[k][:], obsT_d[k * P:(k + 1) * P, c0:c0 + NW])

                hp = new_set()
                um(emb, wd["pw1"], 0, 0, KT, MT,
                   act_evict(hp, bias["pb1"], func=AF.Relu))
                pred = new_set()
                um(hp, wd["pw2"], 0, 0, KT, MT, act_evict(pred, bias["pb2"]))
                ho = new_set()
                um(obs, wd["ow1"], 0, 0, KT, MT,
                   act_evict(ho, bias["ob1"], func=AF.Relu))
                ob2 = new_set()
                um(ho, wd["ow2"], 0, 0, KT, MT, act_evict(ob2, bias["ob2"]))
                for k in range(KT):  # pred <- pred - obs_mapped (in place)
                    nc.vector.tensor_sub(pred[k][:], pred[k][:], ob2[k][:])
                hr = new_set()
                um(pred, wd["rw1"], 0, 0, KT, MT,
                   act_evict(hr, bias["rb1"], func=AF.Relu))
                res = new_set()
                um(hr, wd["rw2"], 0, 0, KT, MT, act_evict(res, bias["rb2"]))

                q = new_set()
                um(emb, wd["wq"], 0, 0, KT, MT, act_evict(q, bias["bq"]))
                kk = new_set()
                um(emb, wd["wk"], 0, 0, KT, MT, act_evict(kk, bias["bk"]))
                v = new_set()
                um(emb, wd["wv"], 0, 0, KT, MT, act_evict(v, bias["bv"]))
                for k in range(KT):  # kk <- K/sqrt(dh) - R  (wk prescaled host-side)
                    nc.vector.tensor_sub(kk[k][:], kk[k][:], res[k][:])

                # ---- attention (per 128-token tile) ----
                attT = new_set()
                for t in range(TT):
                    QA = attbig.tile([P, HID], BF16, tag="QA", name="QA")
                    KA = attbig.tile([P, HID], BF16, tag="KA", name="KA")
                    VA = attbig.tile([P, HID], BF16, tag="VA", name="VA")
                    for f in range(KT):
                        for src, dst in ((q, QA), (kk, KA), (v, VA)):
                            pt = pstr.tile([P, P], BF16, tag="tr", name="tr")
                            nc.tensor.transpose(pt[:], src[f][:, t * P:(t + 1) * P],
                                                ident[:])
                            nc.scalar.activation(dst[:, f * P:(f + 1) * P], pt[:],
                                                 AF.Copy)
                    sc = attp.tile([P, NH * NH], F32, tag="sc", name="sc")
                    prod = attbig.tile([P, HID], F32, tag="prod", name="prod")
                    prod3 = prod[:].rearrange("p (g d) -> p g d", g=NH)
                    prodT = prod[:].rearrange("p (g d) -> p d g", g=NH)
                    KA3 = KA[:].rearrange("p (g d) -> p g d", g=NH)
                    VA3 = VA[:].rearrange("p (g d) -> p g d", g=NH)
                    for h in range(NH):
                        qb = QA[:, h * DH:(h + 1) * DH].unsqueeze(1) \
                            .broadcast_to([P, NH, DH])
                        nc.vector.tensor_tensor(prod3, qb, KA3, op=OP.mult)
                        nc.vector.tensor_reduce(sc[:, h * NH:(h + 1) * NH], prod3,
                                                axis=AX.X, op=OP.add)
                    e = attp.tile([P, NH * NH], BF16, tag="e", name="e")
                    S = attp.tile([P, NH], F32, tag="S", name="S")
                    for h in range(NH):
                        nc.scalar.activation(e[:, h * NH:(h + 1) * NH],
                                             sc[:, h * NH:(h + 1) * NH], AF.Exp,
                                             accum_out=S[:, h:h + 1])
                    rS = attp.tile([P, NH], F32, tag="rS", name="rS")
                    nc.vector.reciprocal(rS[:], S[:])
                    for h in range(NH):
                        eb = e[:, h * NH:(h + 1) * NH].unsqueeze(2) \
                            .broadcast_to([P, NH, DH])
                        nc.vector.tensor_tensor(prod3, eb, VA3, op=OP.mult)
                        att = attp.tile([P, DH], F32, tag="att", name="att")
                        nc.vector.tensor_reduce(att[:], prodT, axis=AX.X, op=OP.add)
                        tmp = attp.tile([P, DH], BF16, tag="tmpb", name="tmpb")
                        nc.vector.tensor_scalar(tmp[:], att[:], rS[:, h:h + 1], None,
                                                op0=OP.mult)
                        pt = pstr.tile([P, P], BF16, tag="tr", name="tr")
                        nc.tensor.transpose(pt[:], tmp[:], ident[:])
                        nc.scalar.activation(attT[h][:, t * P:(t + 1) * P], pt[:],
                                             AF.Copy)

                # ---- wo projection + residual, LN1 ----
                yp = new_set()

                def wo_evict(m, ps):
                    nc.vector.scalar_tensor_tensor(
                        yp[m][:], ps[:], bias["bo"][:, m:m + 1], emb[m][:],
                        op0=OP.add, op1=OP.add)
                um(attT, wd["wo"], 0, 0, KT, MT, wo_evict)
                y1 = new_set()
                layernorm(yp, bias["g1"], bias["be1"], y1)

                # ---- FFN ----
                ffacc = new_set(tag="f", dtype=F32, pool=f32p)
                for j in range(NJ):
                    hf = new_set(tag="a")

                    def f1_evict(m, ps, j=j):
                        c = j * MT + m
                        t1 = actp.tile([P, NW], BF16, tag="a", name="lr1")
                        nc.scalar.activation(t1[:], ps[:], AF.Relu,
                                             bias=fb1a_t[:, c:c + 1], scale=0.99)
                        t2 = actp.tile([P, NW], BF16, tag="a", name="lr2")
                        nc.scalar.activation(t2[:], ps[:], AF.Identity,
                                             bias=fb1b_t[:, c:c + 1], scale=0.01)
                        nc.vector.tensor_add(hf[m][:], t1[:], t2[:])
                    um(y1, fw1_d, 0, j * HID, KT, MT, f1_evict)
                    if j == 0:
                        um(hf, fw2_d, j * HID, 0, KT, MT,
                           act_evict(ffacc, bias["fb2"]))
                    else:
                        def f2_evict(m, ps):
                            nc.vector.tensor_add(ffacc[m][:], ps[:], ffacc[m][:])
                        um(hf, fw2_d, j * HID, 0, KT, MT, f2_evict)

                y2 = new_set()
                for k in range(KT):
                    nc.vector.tensor_add(y2[k][:], y1[k][:], ffacc[k][:])
                out_t = new_set(tag="f", dtype=F32, pool=f32p)
                layernorm(y2, bias["g2"], bias["be2"], out_t)
                for k in range(KT):
                    nc.sync.dma_start(outT_d[k * P:(k + 1) * P, c0:c0 + NW],
                                      out_t[k][:])

    nc.compile()
    return nc


def _pack_bias(b, MT):
    return np.ascontiguousarray(b.reshape(MT, P).T).astype(np.float32)


def make_in_maps(inputs, cfg):
    """Shard FULL inputs into per-core in_maps (host-side prep)."""
    HID, NH, FFN, NTOK = cfg["HID"], cfg["NH"], cfg["FFN"], cfg["NTOK"]
    MT = HID // P
    NJ = FFN // HID
    bf = ml_dtypes.bfloat16

    scale = 1.0 / math.sqrt(HID // NH)
    w_shared = {}
    for n in ["pw1", "pw2", "ow1", "ow2", "rw1", "rw2", "wq", "wv", "wo"]:
        w_shared[n] = np.asarray(inputs[n]).astype(bf)
    w_shared["wk"] = (np.asarray(inputs["wk"]) * scale).astype(bf)
    w_shared["fw1"] = np.asarray(inputs["fw1"]).astype(bf)
    w_shared["fw2"] = np.asarray(inputs["fw2"]).astype(bf)
    b_shared = {}
    for dn, rn in [("pb1", "pb1"), ("pb2", "pb2"), ("ob1", "ob1"),
                   ("ob2", "ob2"), ("rb1", "rb1"), ("rb2", "rb2"),
                   ("bq", "bq"), ("bv", "bv"), ("bo", "bo"), ("fb2", "fb2"),
                   ("g1", "g1"), ("be1", "be1"), ("g2", "g2"), ("be2", "be2")]:
        b_shared[dn] = _pack_bias(np.asarray(inputs[rn], np.float32), MT)
    b_shared["bk"] = _pack_bias(np.asarray(inputs["bk"], np.float32) * scale, MT)
    fb1 = np.asarray(inputs["fb1"], np.float32)
    b_shared["fb1a"] = _pack_bias(fb1 * 0.99, MT * NJ)
    b_shared["fb1b"] = _pack_bias(fb1 * 0.01, MT * NJ)

    emb = np.asarray(inputs["embeddings"], np.float32)
    obsv = np.asarray(inputs["observations"], np.float32)
    in_maps = []
    for c in range(N_CORES):
        r = slice(c * NTOK, (c + 1) * NTOK)
        m = {"embT": np.ascontiguousarray(emb[r].T).astype(bf),
             "obsT": np.ascontiguousarray(obsv[r].T).astype(bf)}
        m.update(w_shared)
        m.update(b_shared)
        in_maps.append(m)
    return in_maps


_NC_CACHE = {}


def get_nc(cfg_key="full"):
    if cfg_key not in _NC_CACHE:
        _NC_CACHE[cfg_key] = build(CFG_FULL, debug=False)
    return _NC_CACHE[cfg_key]


def kernel(**inputs) -> np.ndarray:
    cfg = CFG_FULL
    nc = get_nc()
    in_maps = make_in_maps(inputs, cfg)
    res = None
    for attempt in range(3):
        try:
            res = run_bass_kernel_spmd(nc, in_maps, core_ids=list(range(N_CORES)))
            break
        except Exception:
            if attempt == 2:
                raise
    out = np.empty((N_ROWS, cfg["HID"]), np.float32)
    for c in range(N_CORES):
        r = slice(c * cfg["NTOK"], (c + 1) * cfg["NTOK"])
        out[r] = res.results[c]["outT"].T
    return out
